# revision 14
# baseline (speedup 1.0000x reference)
"""Causal self-attention (B=4, T=2048, C=1024, H=16) on 8 trn2 NeuronCores.

Sharding: tensor-parallel over heads — 2 heads per core.
  phase 1: qkv projection for the local heads (w_attn column-sharded),
           outputs kept transposed ([dim, token]) so attention matmuls
           need no transposes.
  phase 2: causal attention per (batch, head) with scores computed in
           [key, query] layout; softmax denominator comes free from a
           ones-column appended to V; normalization folded into the
           PSUM->SBUF eviction.
  phase 3: AllGather of per-core head outputs (bf16, per-batch chunks).
  phase 4: output projection (w_proj row-permuted + column-sharded);
           each core computes a 128-column slice of the output.
Host only reshapes/casts inputs and concatenates the output slices.
"""

import numpy as np
import ml_dtypes

B, T, C, H = 4, 2048, 1024, 16
DH = C // H          # 64
N_CORES = 8
HPC = H // N_CORES   # heads per core = 2
R = B * T            # 8192 tokens
QCH = 512            # query chunk (phase-2 moving dim)
NQ = T // QCH        # query chunks per batch = 4
KB = 128             # key block
NKB = T // KB        # key blocks per batch = 16

_BF16 = ml_dtypes.bfloat16

_compiled = None     # cached compiled Bass module
LAST_RESULTS = None  # BassKernelResults of the most recent run (for profiling)


def _build():
    import concourse.bass as bass
    import concourse.tile as tile
    from concourse import bacc, mybir
    from concourse.masks import make_identity

    f32 = mybir.dt.float32
    bf16 = mybir.dt.bfloat16
    AF = mybir.ActivationFunctionType

    nc = bacc.Bacc("TRN2", target_bir_lowering=False, debug=False,
                   num_devices=N_CORES)

    xT = nc.dram_tensor("xT", [C, R], bf16, kind="ExternalInput")
    w_qkv = nc.dram_tensor("w_qkv", [C, 3 * HPC * DH], bf16,
                           kind="ExternalInput")
    w_prj = nc.dram_tensor("w_prj", [C, 128], bf16, kind="ExternalInput")
    mask = nc.dram_tensor("mask", [4, KB, QCH], bf16, kind="ExternalInput")
    outT = nc.dram_tensor("outT", [128, R], f32, kind="ExternalOutput")

    with tile.TileContext(nc) as tc:
        with (
            tc.tile_pool(name="const", bufs=1) as const,
            tc.tile_pool(name="xt", bufs=2) as xt_pool,
            tc.tile_pool(name="qkv", bufs=2) as qkv_pool,
            tc.tile_pool(name="vext", bufs=2) as vext_pool,
            tc.tile_pool(name="att", bufs=8) as att_pool,
            tc.tile_pool(name="ysb", bufs=1) as y_pool,
            tc.tile_pool(name="rcp", bufs=3) as rcp_pool,
            tc.tile_pool(name="p4y", bufs=4) as p4y_pool,
            tc.tile_pool(name="osb", bufs=3) as out_pool,
            tc.tile_pool(name="psA", bufs=4, space="PSUM") as psA,
            tc.tile_pool(name="psY", bufs=2, space="PSUM") as psY,
            tc.tile_pool(name="psT", bufs=2, space="PSUM") as psT,
            tc.tile_pool(name="dram", bufs=1, space="DRAM") as dram,
        ):
            # ---- constants ----
            ident = const.tile([128, 128], bf16, tag="ident")
            make_identity(nc, ident[:])

            # const loads on gpsimd queue so the batch-0 x DMAs (sync queue)
            # start immediately
            w_sb = const.tile([128, 8, 3 * HPC * DH], bf16, tag="wqkv")
            for kb in range(8):
                nc.gpsimd.dma_start(w_sb[:, kb, :],
                                    w_qkv[kb * 128:(kb + 1) * 128, :])
            wp_sb = const.tile([128, 8, 128], bf16, tag="wprj")
            for mb in range(8):
                nc.gpsimd.dma_start(wp_sb[:, mb, :],
                                    w_prj[mb * 128:(mb + 1) * 128, :])
            mk_sb = const.tile([128, 4, QCH], bf16, tag="mask")
            for j in range(4):
                nc.gpsimd.dma_start(mk_sb[:, j, :], mask[j])

            y_loc = []   # per-batch DRAM bounce for the collective
            y_gth = []   # per-batch AllGather result (Shared)
            recs_d = []  # per-batch reciprocal row bounce (for bcast reads)
            for b in range(B):
                y_loc.append(dram.tile([128, T], bf16, tag=f"yloc{b}",
                                       name=f"yloc{b}"))
                y_gth.append(dram.tile([N_CORES, 128, T], bf16,
                                       addr_space="Shared", tag=f"ygth{b}",
                                       name=f"ygth{b}"))
                recs_d.append(dram.tile([2 * NQ, QCH], f32, tag=f"recs{b}",
                                        name=f"recs{b}"))

            for b in range(B):
                # ---- phase 1: qkvT[b] = (w_qkv.T @ x[b].T), kept transposed
                xt = xt_pool.tile([128, 8, T], bf16, tag="xt")
                for kb in range(8):
                    nc.sync.dma_start(
                        xt[:, kb, :],
                        xT[kb * 128:(kb + 1) * 128, b * T:(b + 1) * T])
                qkvT = qkv_pool.tile([128, 3, T], bf16, tag="qkvT")
                for rc in range(T // QCH):
                    for m in range(3):
                        ps = psA.tile([128, QCH], f32, tag="mm")
                        for kb in range(8):
                            nc.tensor.matmul(
                                ps[:],
                                w_sb[:, kb, m * 128:(m + 1) * 128],
                                xt[:, kb, rc * QCH:(rc + 1) * QCH],
                                start=(kb == 0), stop=(kb == 7))
                        nc.vector.tensor_copy(
                            qkvT[:, m, rc * QCH:(rc + 1) * QCH], ps[:])

                # ---- vT -> row-major V with a ones column appended per head
                vext = vext_pool.tile([128, NKB, 130], bf16, tag="vext")
                for kb in range(NKB):
                    tr = psT.tile([128, 128], bf16, tag="tr")
                    nc.tensor.transpose(
                        tr[:], qkvT[:, 2, kb * KB:(kb + 1) * KB], ident[:])
                    nc.vector.tensor_copy(vext[:, kb, 0:64], tr[:, 0:64])
                    nc.vector.tensor_copy(vext[:, kb, 65:129], tr[:, 64:128])
                nc.gpsimd.memset(vext[:, :, 64], 1.0)
                nc.gpsimd.memset(vext[:, :, 129], 1.0)

                # ---- phase 2: causal attention, scores in [key, query] layout
                y_sb = y_pool.tile([128, T], bf16, tag=f"y{b}")
                ssb = rcp_pool.tile([2 * NQ, QCH], f32, tag="ssb", bufs=2)
                for q in range(NQ):
                    nkb = 4 * q + 4
                    # score burst: 64x128 row-tiled mode, both heads run
                    # concurrently in the array (h0 rows 0-63, h1 rows 64-127)
                    atts = {}
                    for kb in range(nkb):
                        for h in range(HPC):
                            s_ps = psA.tile([128, QCH], f32, tag="mm")
                            nc.tensor.matmul(
                                s_ps[:],
                                qkvT[64 * h:64 * (h + 1), 1,
                                     kb * KB:(kb + 1) * KB],
                                qkvT[64 * h:64 * (h + 1), 0,
                                     q * QCH:(q + 1) * QCH],
                                start=True, stop=True,
                                tile_position=(64 * h, 0))
                            att = att_pool.tile([128, QCH], bf16, tag="att",
                                                bufs=2 * NKB + 2)
                            nc.scalar.activation(att[:], s_ps[:], AF.Exp,
                                                 scale=0.125)
                            if kb >= 4 * q:  # diagonal block: causal mask
                                j = kb - 4 * q
                                ncol = KB * (j + 1)
                                nc.vector.tensor_mul(
                                    att[:, :ncol], att[:, :ncol],
                                    mk_sb[:, j, :ncol])
                            atts[kb, h] = att
                    # att@v burst: full 128-contraction mode
                    y_pss = {}
                    for h in range(HPC):
                        y_pss[h] = psY.tile([128, QCH], f32, tag="y",
                                            name=f"yps{b}_{q}_{h}")
                    for kb in range(nkb):
                        for h in range(HPC):
                            nc.tensor.matmul(
                                y_pss[h][0:65, :],
                                vext[:, kb, 65 * h:65 * (h + 1)],
                                atts[kb, h][:],
                                start=(kb == 0), stop=(kb == nkb - 1))
                    for h in range(HPC):
                        y_ps = y_pss[h]
                        # stash unnormalized y + sums row; normalize later
                        nc.vector.tensor_copy(
                            y_sb[64 * h:64 * (h + 1),
                                 q * QCH:(q + 1) * QCH],
                            y_ps[0:64, :])
                        srow = rcp_pool.tile([128, QCH], f32, tag="srow", bufs=2)
                        nc.vector.tensor_copy(srow[0:1, :], y_ps[64:65, :])
                        r = 2 * q + h
                        nc.sync.dma_start(ssb[r:r + 1, :], srow[0:1, :])

                # one reciprocal for the whole batch, broadcast via DRAM read
                rsb = rcp_pool.tile([2 * NQ, QCH], f32, tag="rsb", bufs=2)
                nc.vector.reciprocal(rsb[:], ssb[:])
                nc.sync.dma_start(recs_d[b][:], rsb[:])
                bc64 = rcp_pool.tile([128, 2 * NQ, QCH], f32, tag="bc64", bufs=1)
                nc.sync.dma_start(
                    bc64[:], recs_d[b][None, :, :].broadcast_to(
                        [128, 2 * NQ, QCH]))
                for q in range(NQ):
                    for h in range(HPC):
                        r = 2 * q + h
                        ysl = y_sb[64 * h:64 * (h + 1),
                                   q * QCH:(q + 1) * QCH]
                        nc.vector.tensor_mul(
                            ysl, ysl, bc64[64 * h:64 * (h + 1), r, :])

                # ---- phase 3: publish local heads, gather all heads
                nc.sync.dma_start(y_loc[b][:], y_sb[:])
                nc.gpsimd.collective_compute(
                    "AllGather", bass.mybir.AluOpType.bypass,
                    replica_groups=[list(range(N_CORES))],
                    ins=[y_loc[b].opt()], outs=[y_gth[b].opt()])

            # ---- phase 4 (all batches last, so AllGather waits overlap
            #      with later batches' compute): output projection
            for b in range(B):
                for rc in range(T // QCH):
                    ps = psA.tile([128, QCH], f32, tag="mm")
                    for mb in range(8):
                        yg = p4y_pool.tile([128, QCH], bf16, tag="p4y")
                        nc.sync.dma_start(
                            yg[:], y_gth[b][mb, :, rc * QCH:(rc + 1) * QCH])
                        nc.tensor.matmul(ps[:], wp_sb[:, mb, :], yg[:],
                                         start=(mb == 0), stop=(mb == 7))
                    osb = out_pool.tile([128, QCH], f32, tag="osb")
                    nc.vector.tensor_copy(osb[:], ps[:])
                    nc.sync.dma_start(
                        outT[:, b * T + rc * QCH: b * T + (rc + 1) * QCH],
                        osb[:])

    nc.compile()
    return nc


def kernel(x: np.ndarray, w_attn: np.ndarray, w_proj: np.ndarray) -> np.ndarray:
    global _compiled, LAST_RESULTS
    from concourse.bass_utils import run_bass_kernel_spmd

    assert x.shape == (B, T, C) and w_attn.shape == (C, 3 * C)
    assert w_proj.shape == (C, C)

    # ---- host-side shard prep (cast + layout only) ----
    xT = np.ascontiguousarray(x.reshape(R, C).T).astype(_BF16)

    # per-core w_attn column slice, columns ordered [q_h0|q_h1|k_h0|k_h1|v_h0|v_h1]
    w_qkv_c = []
    for c in range(N_CORES):
        cols = []
        for sec in range(3):                      # q, k, v sections
            for h in (HPC * c, HPC * c + 1):
                base = sec * C + h * DH
                cols.append(np.arange(base, base + DH))
        w_qkv_c.append(np.ascontiguousarray(
            w_attn[:, np.concatenate(cols)]).astype(_BF16))

    # w_proj rows permuted to the gathered-y ordering, then column-sharded
    perm = np.concatenate([
        np.arange((HPC * c + h) * DH, (HPC * c + h + 1) * DH)
        for c in range(N_CORES) for h in range(HPC)])
    w_proj_perm = w_proj[perm, :]
    w_prj_c = [np.ascontiguousarray(
        w_proj_perm[:, 128 * c:128 * (c + 1)]).astype(_BF16)
        for c in range(N_CORES)]

    # multiplicative causal masks for the 4 diagonal key-blocks of a q-chunk
    kk = np.arange(KB)[:, None]
    qq = np.arange(QCH)[None, :]
    mask = np.stack([(kk + KB * j <= qq) for j in range(4)]).astype(_BF16)

    if _compiled is None:
        _compiled = _build()

    in_maps = [{"xT": xT, "w_qkv": w_qkv_c[c], "w_prj": w_prj_c[c],
                "mask": mask} for c in range(N_CORES)]
    LAST_RESULTS = run_bass_kernel_spmd(_compiled, in_maps,
                                        list(range(N_CORES)))

    out = np.empty((R, C), dtype=np.float32)
    for c in range(N_CORES):
        out[:, 128 * c:128 * (c + 1)] = LAST_RESULTS.results[c]["outT"].T
    return out.reshape(B, T, C)


# revision 16
# speedup vs baseline: 1.2261x; 1.2261x over previous
"""Causal self-attention (B=4, T=2048, C=1024, H=16) on 8 trn2 NeuronCores.

Sharding: tensor-parallel over heads — 2 heads per core.
  phase 1: qkv projection for the local heads (w_attn column-sharded),
           outputs kept transposed ([dim, token]) so attention matmuls
           need no transposes.
  phase 2: causal attention per (batch, head) with scores computed in
           [key, query] layout; softmax denominator comes free from a
           ones-column appended to V; normalization folded into the
           PSUM->SBUF eviction.
  phase 3: AllGather of per-core head outputs (bf16, per-batch chunks).
  phase 4: output projection (w_proj row-permuted + column-sharded);
           each core computes a 128-column slice of the output.
Host only reshapes/casts inputs and concatenates the output slices.
"""

import numpy as np
import ml_dtypes

B, T, C, H = 4, 2048, 1024, 16
DH = C // H          # 64
N_CORES = 8
HPC = H // N_CORES   # heads per core = 2
R = B * T            # 8192 tokens
QCH = 512            # query chunk (phase-2 moving dim)
NQ = T // QCH        # query chunks per batch = 4
KB = 128             # key block
NKB = T // KB        # key blocks per batch = 16

_BF16 = ml_dtypes.bfloat16

_compiled = None     # cached compiled Bass module
LAST_RESULTS = None  # BassKernelResults of the most recent run (for profiling)


def _build():
    import concourse.bass as bass
    import concourse.tile as tile
    from concourse import bacc, mybir
    from concourse.masks import make_identity

    f32 = mybir.dt.float32
    bf16 = mybir.dt.bfloat16
    AF = mybir.ActivationFunctionType

    nc = bacc.Bacc("TRN2", target_bir_lowering=False, debug=False,
                   num_devices=N_CORES)

    xT = nc.dram_tensor("xT", [C, R], bf16, kind="ExternalInput")
    w_qkv = nc.dram_tensor("w_qkv", [C, 3 * HPC * DH], bf16,
                           kind="ExternalInput")
    w_prj = nc.dram_tensor("w_prj", [C, 128], bf16, kind="ExternalInput")
    mask = nc.dram_tensor("mask", [4, KB, QCH], bf16, kind="ExternalInput")
    outT = nc.dram_tensor("outT", [128, R], f32, kind="ExternalOutput")

    with tile.TileContext(nc) as tc:
        with (
            tc.tile_pool(name="const", bufs=1) as const,
            tc.tile_pool(name="xt", bufs=2) as xt_pool,
            tc.tile_pool(name="qkv", bufs=2) as qkv_pool,
            tc.tile_pool(name="vext", bufs=2) as vext_pool,
            tc.tile_pool(name="att", bufs=8) as att_pool,
            tc.tile_pool(name="ysb", bufs=1) as y_pool,
            tc.tile_pool(name="rcp", bufs=3) as rcp_pool,
            tc.tile_pool(name="p4y", bufs=4) as p4y_pool,
            tc.tile_pool(name="osb", bufs=3) as out_pool,
            tc.tile_pool(name="psA", bufs=2, space="PSUM") as psA,
            tc.tile_pool(name="psY", bufs=2, space="PSUM") as psY,
            tc.tile_pool(name="dram", bufs=1, space="DRAM") as dram,
        ):
            # ---- constants ----
            ident = const.tile([128, 128], bf16, tag="ident")
            make_identity(nc, ident[:])

            # const loads on gpsimd queue so the batch-0 x DMAs (sync queue)
            # start immediately
            w_sb = const.tile([128, 8, 3 * HPC * DH], bf16, tag="wqkv")
            for kb in range(8):
                nc.gpsimd.dma_start(w_sb[:, kb, :],
                                    w_qkv[kb * 128:(kb + 1) * 128, :])
            wp_sb = const.tile([128, 8, 128], bf16, tag="wprj")
            for mb in range(8):
                nc.gpsimd.dma_start(wp_sb[:, mb, :],
                                    w_prj[mb * 128:(mb + 1) * 128, :])
            mk_sb = const.tile([128, 4, QCH], bf16, tag="mask")
            for j in range(4):
                nc.gpsimd.dma_start(mk_sb[:, j, :], mask[j])

            y_loc = []   # per-batch DRAM bounce for the collective
            y_gth = []   # per-batch AllGather result (Shared)
            recs_d = []  # per-batch reciprocal row bounce (for bcast reads)
            for b in range(B):
                y_loc.append(dram.tile([128, T], bf16, tag=f"yloc{b}",
                                       name=f"yloc{b}"))
                y_gth.append(dram.tile([N_CORES, 128, T], bf16,
                                       addr_space="Shared", tag=f"ygth{b}",
                                       name=f"ygth{b}"))
                recs_d.append(dram.tile([2 * NQ, QCH], f32, tag=f"recs{b}",
                                        name=f"recs{b}"))

            for b in range(B):
                # ---- phase 1: qkvT[b] = (w_qkv.T @ x[b].T), kept transposed
                xt = xt_pool.tile([128, 8, T], bf16, tag="xt")
                for kb in range(8):
                    nc.sync.dma_start(
                        xt[:, kb, :],
                        xT[kb * 128:(kb + 1) * 128, b * T:(b + 1) * T])
                qkvT = qkv_pool.tile([128, 3, T], bf16, tag="qkvT")
                for rc in range(T // QCH):
                    for m in range(3):
                        ps = psA.tile([128, QCH], f32, tag="mm1")
                        for kb in range(8):
                            nc.tensor.matmul(
                                ps[:],
                                w_sb[:, kb, m * 128:(m + 1) * 128],
                                xt[:, kb, rc * QCH:(rc + 1) * QCH],
                                start=(kb == 0), stop=(kb == 7))
                        nc.vector.tensor_copy(
                            qkvT[:, m, rc * QCH:(rc + 1) * QCH], ps[:])

                # ---- vT -> row-major V with a ones column appended per head
                vext = vext_pool.tile([128, NKB, 130], bf16, tag="vext")
                for kb in range(NKB):
                    tr = psY.tile([128, 128], bf16, tag="y", name=f"tr{b}_{kb}")
                    nc.tensor.transpose(
                        tr[:], qkvT[:, 2, kb * KB:(kb + 1) * KB], ident[:])
                    nc.vector.tensor_copy(vext[:, kb, 0:64], tr[:, 0:64])
                    nc.vector.tensor_copy(vext[:, kb, 65:129], tr[:, 64:128])
                nc.gpsimd.memset(vext[:, :, 64], 1.0)
                nc.gpsimd.memset(vext[:, :, 129], 1.0)

                # ---- phase 2: causal attention, scores in [key, query] layout
                y_sb = y_pool.tile([128, T], bf16, tag=f"y{b}")
                ssb = rcp_pool.tile([2 * NQ, QCH], f32, tag="ssb", bufs=2)
                for q in range(NQ):
                    nkb = 4 * q + 4
                    for h in range(HPC):
                        qT = qkvT[64 * h:64 * (h + 1), 0,
                                  q * QCH:(q + 1) * QCH]
                        y_ps = psY.tile([128, QCH], f32, tag="y")
                        # two key-blocks share one psum tile so a single
                        # exp covers [128, 1024] (amortizes ACT overhead)
                        for kp in range(nkb // 2):
                            s_ps = psA.tile([128, 2, QCH], f32, tag="mm")
                            att = att_pool.tile([128, 2, QCH], bf16,
                                                tag="att")
                            for half in range(2):
                                kb = 2 * kp + half
                                nc.tensor.matmul(
                                    s_ps[:, half, :],
                                    qkvT[64 * h:64 * (h + 1), 1,
                                         kb * KB:(kb + 1) * KB],
                                    qT, start=True, stop=True)
                            nc.scalar.activation(att[:], s_ps[:], AF.Exp,
                                                 scale=0.125)
                            for half in range(2):
                                kb = 2 * kp + half
                                if kb >= 4 * q:  # diagonal: causal mask
                                    j = kb - 4 * q
                                    ncol = KB * (j + 1)
                                    nc.vector.tensor_mul(
                                        att[:, half, :ncol],
                                        att[:, half, :ncol],
                                        mk_sb[:, j, :ncol])
                                nc.tensor.matmul(
                                    y_ps[0:65, :],
                                    vext[:, kb, 65 * h:65 * (h + 1)],
                                    att[:, half, :],
                                    start=(kb == 0), stop=(kb == nkb - 1))
                        # stash unnormalized y + sums row; normalize later
                        nc.vector.tensor_copy(
                            y_sb[64 * h:64 * (h + 1),
                                 q * QCH:(q + 1) * QCH],
                            y_ps[0:64, :])
                        srow = rcp_pool.tile([128, QCH], f32, tag="srow", bufs=2)
                        nc.vector.tensor_copy(srow[0:1, :], y_ps[64:65, :])
                        r = 2 * q + h
                        nc.sync.dma_start(ssb[r:r + 1, :], srow[0:1, :])

                # one reciprocal for the whole batch, broadcast via DRAM read
                rsb = rcp_pool.tile([2 * NQ, QCH], f32, tag="rsb", bufs=2)
                nc.vector.reciprocal(rsb[:], ssb[:])
                nc.sync.dma_start(recs_d[b][:], rsb[:])
                bc64 = rcp_pool.tile([128, 2 * NQ, QCH], f32, tag="bc64", bufs=1)
                nc.sync.dma_start(
                    bc64[:], recs_d[b][None, :, :].broadcast_to(
                        [128, 2 * NQ, QCH]))
                for q in range(NQ):
                    for h in range(HPC):
                        r = 2 * q + h
                        ysl = y_sb[64 * h:64 * (h + 1),
                                   q * QCH:(q + 1) * QCH]
                        nc.vector.tensor_mul(
                            ysl, ysl, bc64[64 * h:64 * (h + 1), r, :])

                # ---- phase 3: publish local heads, gather all heads
                nc.sync.dma_start(y_loc[b][:], y_sb[:])
                nc.gpsimd.collective_compute(
                    "AllGather", bass.mybir.AluOpType.bypass,
                    replica_groups=[list(range(N_CORES))],
                    ins=[y_loc[b].opt()], outs=[y_gth[b].opt()])

            # ---- phase 4 (all batches last, so AllGather waits overlap
            #      with later batches' compute): output projection
            for b in range(B):
                for rc in range(T // QCH):
                    ps = psA.tile([128, QCH], f32, tag="mm1")
                    for mb in range(8):
                        yg = p4y_pool.tile([128, QCH], bf16, tag="p4y")
                        nc.sync.dma_start(
                            yg[:], y_gth[b][mb, :, rc * QCH:(rc + 1) * QCH])
                        nc.tensor.matmul(ps[:], wp_sb[:, mb, :], yg[:],
                                         start=(mb == 0), stop=(mb == 7))
                    osb = out_pool.tile([128, QCH], f32, tag="osb")
                    nc.vector.tensor_copy(osb[:], ps[:])
                    nc.sync.dma_start(
                        outT[:, b * T + rc * QCH: b * T + (rc + 1) * QCH],
                        osb[:])

    nc.compile()
    return nc


def kernel(x: np.ndarray, w_attn: np.ndarray, w_proj: np.ndarray) -> np.ndarray:
    global _compiled, LAST_RESULTS
    from concourse.bass_utils import run_bass_kernel_spmd

    assert x.shape == (B, T, C) and w_attn.shape == (C, 3 * C)
    assert w_proj.shape == (C, C)

    # ---- host-side shard prep (cast + layout only) ----
    xT = np.ascontiguousarray(x.reshape(R, C).T).astype(_BF16)

    # per-core w_attn column slice, columns ordered [q_h0|q_h1|k_h0|k_h1|v_h0|v_h1]
    w_qkv_c = []
    for c in range(N_CORES):
        cols = []
        for sec in range(3):                      # q, k, v sections
            for h in (HPC * c, HPC * c + 1):
                base = sec * C + h * DH
                cols.append(np.arange(base, base + DH))
        w_qkv_c.append(np.ascontiguousarray(
            w_attn[:, np.concatenate(cols)]).astype(_BF16))

    # w_proj rows permuted to the gathered-y ordering, then column-sharded
    perm = np.concatenate([
        np.arange((HPC * c + h) * DH, (HPC * c + h + 1) * DH)
        for c in range(N_CORES) for h in range(HPC)])
    w_proj_perm = w_proj[perm, :]
    w_prj_c = [np.ascontiguousarray(
        w_proj_perm[:, 128 * c:128 * (c + 1)]).astype(_BF16)
        for c in range(N_CORES)]

    # multiplicative causal masks for the 4 diagonal key-blocks of a q-chunk
    kk = np.arange(KB)[:, None]
    qq = np.arange(QCH)[None, :]
    mask = np.stack([(kk + KB * j <= qq) for j in range(4)]).astype(_BF16)

    if _compiled is None:
        _compiled = _build()

    in_maps = [{"xT": xT, "w_qkv": w_qkv_c[c], "w_prj": w_prj_c[c],
                "mask": mask} for c in range(N_CORES)]
    LAST_RESULTS = run_bass_kernel_spmd(_compiled, in_maps,
                                        list(range(N_CORES)))

    out = np.empty((R, C), dtype=np.float32)
    for c in range(N_CORES):
        out[:, 128 * c:128 * (c + 1)] = LAST_RESULTS.results[c]["outT"].T
    return out.reshape(B, T, C)
